# revision 16
# baseline (speedup 1.0000x reference)
"""Block-diagonal complex matmul kernel for trn2 (8 NeuronCores).

Reference computation:
  xp = take(x, perm_idx, axis=-2).reshape(B, 2, M, S)
  y_re = xp_re @ hr1 + xp_im @ hi1   (per block a of M)
  y_im = xp_re @ hi2 + xp_im @ hr2
  out  = stack([y_re, y_im], 1).reshape(B, 2, N, R)

Sharding: block dim M=1024 split across 8 cores (128 blocks each).
Permutation gather + all layout shuffles happen host-side in numpy.

Precision: weights are quantized to fp8 e3m4 (scaled x16 into the format's
normal range), x carries the inverse scale in fp16 (exact, power of two).
Measured end-to-end rel err 1.33e-2 against the fp32 reference (gate 2e-2).

Per-core device kernel, per block a:
  psum[16, 256]  = x_re[:, a].T @ [hr1[a] | hi2[a]]   (start)
                 + x_im[:, a].T @ [hi1[a] | hr2[a]]   (stop)
  -> cols 0:128 = y_re[a], cols 128:256 = y_im[a]; evicted to fp16 on
  alternating DVE/ACT.

Queue discipline: weight loads run alone on the Sync DGE queue (pure
stream, deep prefetch); y stores go on the Scalar DGE queue so a store
waiting on evictions can never block weight prefetch.
"""

import os
import numpy as np

B = 16
N = 4096
R = 32
M = 1024   # blocks
S = 128    # block size (contract dim)
NCORES = 8
MLOC = M // NCORES   # 128 blocks per core
# weight DMA group sizes in blocks: small-ish first group so the first
# matmul's dependencies arrive early; small last groups to shorten the
# tail. Segments stay >= 2 KiB/partition for DMA packet efficiency.
WGRPS = [8] * 16
# x chunk sizes in blocks (per re/im tensor); interleaved into the sync
# queue after the matching early w groups
XCHUNKS = [32, 32, 32, 32]
WSCALE = 16.0        # host multiplies W by this, divides x by it (exact)


def _prefix(sizes):
    out = [0]
    for n in sizes:
        out.append(out[-1] + n)
    return out


WG0 = _prefix(WGRPS)
XC0 = _prefix(XCHUNKS)
assert WG0[-1] == MLOC and XC0[-1] == MLOC
assert all(n % 2 == 0 for n in WGRPS)


def _xchunk_of(a):
    for c in range(len(XCHUNKS)):
        if a < XC0[c + 1]:
            return c
    raise AssertionError(a)


_NC_CACHE = {}


def _build_nc():
    import concourse.bacc as bacc
    import concourse.bass as bass
    import concourse.mybir as mybir
    from concourse import tile

    x_dt = mybir.dt.float16
    w_dt = mybir.dt.float8e3
    y_dt = mybir.dt.float16
    nc = bacc.Bacc(None, target_bir_lowering=False)

    xrp = nc.dram_tensor("xrp", [S, MLOC * B], x_dt, kind="ExternalInput")
    xip = nc.dram_tensor("xip", [S, MLOC * B], x_dt, kind="ExternalInput")
    # weights: per block 512 fp8 cols = [W1 | W2]
    # with W1 = [hr1 | hi2], W2 = [hi1 | hr2]  (x16 in e3m4)
    WC = 4 * S  # 512 cols per block
    w = nc.dram_tensor("w", [S, MLOC * WC], w_dt, kind="ExternalInput")
    y = nc.dram_tensor("y", [B, MLOC * 2 * S], y_dt, kind="ExternalOutput")

    with tile.TileContext(nc) as tc:
        with (
            tc.tile_pool(name="xp", bufs=1) as xpool,
            tc.tile_pool(name="wp", bufs=8) as wpool,
            tc.tile_pool(name="op", bufs=4) as opool,
            tc.tile_pool(name="ps", bufs=8, space=bass.MemorySpace.PSUM) as ps,
        ):
            # single load stream on the sync queue: w0, x0, w1, x1, ...,
            # then the remaining w groups. Stores live on the scalar queue
            # so they can never block weight prefetch.
            xr_t = [None] * len(XCHUNKS)
            xi_t = [None] * len(XCHUNKS)
            wts = []
            for g, nb in enumerate(WGRPS):
                wt = wpool.tile([S, nb * WC], w_dt)
                wq = nc.sync if g % 2 == 0 else nc.scalar
                wq.dma_start(wt[:], w[:, WG0[g] * WC:WG0[g + 1] * WC])
                wts.append(wt)
                if g < len(XCHUNKS):
                    cw = XCHUNKS[g] * B
                    xr_t[g] = xpool.tile([S, cw], x_dt, name=f"xr{g}")
                    xi_t[g] = xpool.tile([S, cw], x_dt, name=f"xi{g}")
                    nc.sync.dma_start(xr_t[g][:], xrp[:, XC0[g] * B:XC0[g + 1] * B])
                    nc.sync.dma_start(xi_t[g][:], xip[:, XC0[g] * B:XC0[g + 1] * B])
            npair = 0
            for g, nb in enumerate(WGRPS):
                wt = wts[g]
                last2 = g >= len(WGRPS) - 2
                ot = opool.tile([B, nb * 2 * S], y_dt)
                for i in range(0, nb, 2):
                    # two blocks share one full PSUM bank
                    pt = ps.tile([B, 4 * S], mybir.dt.float32)
                    for j in range(2):
                        a = WG0[g] + i + j
                        c0 = (i + j) * WC
                        w1 = wt[:, c0:c0 + 2 * S]
                        w2 = wt[:, c0 + 2 * S:c0 + 4 * S]
                        xc = _xchunk_of(a)
                        xo = a - XC0[xc]
                        xs = slice(xo * B, (xo + 1) * B)
                        psl = pt[:, j * 2 * S:(j + 1) * 2 * S]
                        nc.tensor.matmul(psl, xr_t[xc][:, xs], w1, start=True, stop=False)
                        nc.tensor.matmul(psl, xi_t[xc][:, xs], w2, start=False, stop=True)
                    osl = ot[:, i * 2 * S:(i + 2) * 2 * S]
                    if npair % 2 == 0:
                        nc.vector.tensor_copy(osl, pt[:])
                    else:
                        nc.scalar.copy(osl, pt[:])
                    npair += 1
                nc.gpsimd.dma_start(y[:, WG0[g] * 2 * S:WG0[g + 1] * 2 * S], ot[:])
    nc.compile()
    return nc


def kernel(x, hr1, hi1, hr2, hi2, perm_idx):
    import ml_dtypes
    from concourse.bass_utils import run_bass_kernel_spmd

    if "nc" not in _NC_CACHE:
        _NC_CACHE["nc"] = _build_nc()
    nc = _NC_CACHE["nc"]

    f8 = ml_dtypes.float8_e3m4
    x = np.asarray(x, dtype=np.float32)
    perm_idx = np.asarray(perm_idx)
    # host-side permutation gather + regroup into M blocks of size S
    xp = x[:, :, perm_idx, :].reshape(B, 2, M, S) * np.float32(1.0 / WSCALE)
    xp16 = xp.astype(np.float16)

    in_maps = []
    for c in range(NCORES):
        a0 = c * MLOC
        sl = slice(a0, a0 + MLOC)
        # [B, MLOC, S] -> [S(j), MLOC, B] -> [S, MLOC*B]
        xre = np.ascontiguousarray(
            np.transpose(xp16[:, 0, sl, :], (2, 1, 0))
        ).reshape(S, MLOC * B)
        xim = np.ascontiguousarray(
            np.transpose(xp16[:, 1, sl, :], (2, 1, 0))
        ).reshape(S, MLOC * B)
        # W1 = [hr1 | hi2], W2 = [hi1 | hr2]; per block [W1 | W2] x16 in e3m4
        wc = np.concatenate(
            [hr1[sl], hi2[sl], hi1[sl], hr2[sl]], axis=2
        ) * np.float32(WSCALE)
        wc = wc.astype(f8)  # [MLOC, S, 4S]
        wc = np.ascontiguousarray(np.transpose(wc, (1, 0, 2))).reshape(S, MLOC * 4 * S)
        in_maps.append({"xrp": xre, "xip": xim, "w": wc})

    trace = bool(os.environ.get("KERNEL_TRACE"))
    kwargs = {}
    if trace:
        kwargs["tmpdir"] = os.environ.get("KERNEL_TRACE_DIR") or None
    res = run_bass_kernel_spmd(nc, in_maps, core_ids=list(range(NCORES)), trace=trace, **kwargs)
    if trace and res.exec_time_ns is not None:
        print(f"HW exec time: {res.exec_time_ns} ns")
        _NC_CACHE["exec_time_ns"] = res.exec_time_ns
        _NC_CACHE["profile"] = res

    out = np.empty((B, 2, M, S), dtype=np.float32)
    for c in range(NCORES):
        a0 = c * MLOC
        yc = res.results[c]["y"].astype(np.float32).reshape(B, MLOC, 2, S)
        out[:, 0, a0:a0 + MLOC, :] = yc[:, :, 0, :]
        out[:, 1, a0:a0 + MLOC, :] = yc[:, :, 1, :]
    return out.reshape(B, 2, N, R)


# revision 17
# speedup vs baseline: 1.0259x; 1.0259x over previous
"""Block-diagonal complex matmul kernel for trn2 (8 NeuronCores).

Reference computation:
  xp = take(x, perm_idx, axis=-2).reshape(B, 2, M, S)
  y_re = xp_re @ hr1 + xp_im @ hi1   (per block a of M)
  y_im = xp_re @ hi2 + xp_im @ hr2
  out  = stack([y_re, y_im], 1).reshape(B, 2, N, R)

Sharding: block dim M=1024 split across 8 cores (128 blocks each).
Permutation gather + all layout shuffles happen host-side in numpy.

Precision: weights are quantized to fp8 e3m4 (scaled x16 into the format's
normal range), x carries the inverse scale in fp16 (exact, power of two).
Measured end-to-end rel err 1.33e-2 against the fp32 reference (gate 2e-2).

Per-core device kernel, per block a:
  psum[16, 256]  = x_re[:, a].T @ [hr1[a] | hi2[a]]   (start)
                 + x_im[:, a].T @ [hi1[a] | hr2[a]]   (stop)
  -> cols 0:128 = y_re[a], cols 128:256 = y_im[a]; evicted to fp16 on
  alternating DVE/ACT.

Queue discipline: weight loads run alone on the Sync DGE queue (pure
stream, deep prefetch); y stores go on the Scalar DGE queue so a store
waiting on evictions can never block weight prefetch.
"""

import os
import numpy as np

B = 16
N = 4096
R = 32
M = 1024   # blocks
S = 128    # block size (contract dim)
NCORES = 8
MLOC = M // NCORES   # 128 blocks per core
# weight DMA group sizes in blocks: small-ish first group so the first
# matmul's dependencies arrive early; small last groups to shorten the
# tail. Segments stay >= 2 KiB/partition for DMA packet efficiency.
WGRPS = [4] + [8] * 15 + [4]
# x chunk sizes in blocks (per re/im tensor); interleaved into the sync
# queue after the matching early w groups
XCHUNKS = [16, 16, 32, 32, 32]
WSCALE = 16.0        # host multiplies W by this, divides x by it (exact)


def _prefix(sizes):
    out = [0]
    for n in sizes:
        out.append(out[-1] + n)
    return out


WG0 = _prefix(WGRPS)
XC0 = _prefix(XCHUNKS)
assert WG0[-1] == MLOC and XC0[-1] == MLOC
assert all(n % 2 == 0 for n in WGRPS)


def _xchunk_of(a):
    for c in range(len(XCHUNKS)):
        if a < XC0[c + 1]:
            return c
    raise AssertionError(a)


_NC_CACHE = {}


def _build_nc():
    import concourse.bacc as bacc
    import concourse.bass as bass
    import concourse.mybir as mybir
    from concourse import tile

    x_dt = mybir.dt.float16
    w_dt = mybir.dt.float8e3
    y_dt = mybir.dt.float16
    nc = bacc.Bacc(None, target_bir_lowering=False)

    xrp = nc.dram_tensor("xrp", [S, MLOC * B], x_dt, kind="ExternalInput")
    xip = nc.dram_tensor("xip", [S, MLOC * B], x_dt, kind="ExternalInput")
    # weights: per block 512 fp8 cols = [W1 | W2]
    # with W1 = [hr1 | hi2], W2 = [hi1 | hr2]  (x16 in e3m4)
    WC = 4 * S  # 512 cols per block
    w = nc.dram_tensor("w", [S, MLOC * WC], w_dt, kind="ExternalInput")
    y = nc.dram_tensor("y", [B, MLOC * 2 * S], y_dt, kind="ExternalOutput")

    with tile.TileContext(nc) as tc:
        with (
            tc.tile_pool(name="xp", bufs=1) as xpool,
            tc.tile_pool(name="wp", bufs=8) as wpool,
            tc.tile_pool(name="op", bufs=4) as opool,
            tc.tile_pool(name="ps", bufs=8, space=bass.MemorySpace.PSUM) as ps,
        ):
            # single load stream on the sync queue: w0, x0, w1, x1, ...,
            # then the remaining w groups. Stores live on the scalar queue
            # so they can never block weight prefetch.
            xr_t = [None] * len(XCHUNKS)
            xi_t = [None] * len(XCHUNKS)
            wts = []
            for g, nb in enumerate(WGRPS):
                wt = wpool.tile([S, nb * WC], w_dt)
                nc.sync.dma_start(wt[:], w[:, WG0[g] * WC:WG0[g + 1] * WC])
                wts.append(wt)
                if g < len(XCHUNKS):
                    cw = XCHUNKS[g] * B
                    xr_t[g] = xpool.tile([S, cw], x_dt, name=f"xr{g}")
                    xi_t[g] = xpool.tile([S, cw], x_dt, name=f"xi{g}")
                    nc.sync.dma_start(xr_t[g][:], xrp[:, XC0[g] * B:XC0[g + 1] * B])
                    nc.sync.dma_start(xi_t[g][:], xip[:, XC0[g] * B:XC0[g + 1] * B])
            npair = 0
            for g, nb in enumerate(WGRPS):
                wt = wts[g]
                last2 = g >= len(WGRPS) - 2
                ot = opool.tile([B, nb * 2 * S], y_dt)
                for i in range(0, nb, 2):
                    # two blocks share one full PSUM bank
                    pt = ps.tile([B, 4 * S], mybir.dt.float32)
                    for j in range(2):
                        a = WG0[g] + i + j
                        c0 = (i + j) * WC
                        w1 = wt[:, c0:c0 + 2 * S]
                        w2 = wt[:, c0 + 2 * S:c0 + 4 * S]
                        xc = _xchunk_of(a)
                        xo = a - XC0[xc]
                        xs = slice(xo * B, (xo + 1) * B)
                        psl = pt[:, j * 2 * S:(j + 1) * 2 * S]
                        nc.tensor.matmul(psl, xr_t[xc][:, xs], w1, start=True, stop=False)
                        nc.tensor.matmul(psl, xi_t[xc][:, xs], w2, start=False, stop=True)
                    osl = ot[:, i * 2 * S:(i + 2) * 2 * S]
                    if npair % 2 == 0:
                        nc.vector.tensor_copy(osl, pt[:])
                    else:
                        nc.scalar.copy(osl, pt[:])
                    npair += 1
                nc.scalar.dma_start(y[:, WG0[g] * 2 * S:WG0[g + 1] * 2 * S], ot[:])
    nc.compile()
    return nc


def kernel(x, hr1, hi1, hr2, hi2, perm_idx):
    import ml_dtypes
    from concourse.bass_utils import run_bass_kernel_spmd

    if "nc" not in _NC_CACHE:
        _NC_CACHE["nc"] = _build_nc()
    nc = _NC_CACHE["nc"]

    f8 = ml_dtypes.float8_e3m4
    x = np.asarray(x, dtype=np.float32)
    perm_idx = np.asarray(perm_idx)
    # host-side permutation gather + regroup into M blocks of size S
    xp = x[:, :, perm_idx, :].reshape(B, 2, M, S) * np.float32(1.0 / WSCALE)
    xp16 = xp.astype(np.float16)

    in_maps = []
    for c in range(NCORES):
        a0 = c * MLOC
        sl = slice(a0, a0 + MLOC)
        # [B, MLOC, S] -> [S(j), MLOC, B] -> [S, MLOC*B]
        xre = np.ascontiguousarray(
            np.transpose(xp16[:, 0, sl, :], (2, 1, 0))
        ).reshape(S, MLOC * B)
        xim = np.ascontiguousarray(
            np.transpose(xp16[:, 1, sl, :], (2, 1, 0))
        ).reshape(S, MLOC * B)
        # W1 = [hr1 | hi2], W2 = [hi1 | hr2]; per block [W1 | W2] x16 in e3m4
        wc = np.concatenate(
            [hr1[sl], hi2[sl], hi1[sl], hr2[sl]], axis=2
        ) * np.float32(WSCALE)
        wc = wc.astype(f8)  # [MLOC, S, 4S]
        wc = np.ascontiguousarray(np.transpose(wc, (1, 0, 2))).reshape(S, MLOC * 4 * S)
        in_maps.append({"xrp": xre, "xip": xim, "w": wc})

    trace = bool(os.environ.get("KERNEL_TRACE"))
    kwargs = {}
    if trace:
        kwargs["tmpdir"] = os.environ.get("KERNEL_TRACE_DIR") or None
    res = run_bass_kernel_spmd(nc, in_maps, core_ids=list(range(NCORES)), trace=trace, **kwargs)
    if trace and res.exec_time_ns is not None:
        print(f"HW exec time: {res.exec_time_ns} ns")
        _NC_CACHE["exec_time_ns"] = res.exec_time_ns
        _NC_CACHE["profile"] = res

    out = np.empty((B, 2, M, S), dtype=np.float32)
    for c in range(NCORES):
        a0 = c * MLOC
        yc = res.results[c]["y"].astype(np.float32).reshape(B, MLOC, 2, S)
        out[:, 0, a0:a0 + MLOC, :] = yc[:, :, 0, :]
        out[:, 1, a0:a0 + MLOC, :] = yc[:, :, 1, :]
    return out.reshape(B, 2, N, R)


# revision 18
# speedup vs baseline: 1.1785x; 1.1487x over previous
"""Block-diagonal complex matmul kernel for trn2 (8 NeuronCores).

Reference computation:
  xp = take(x, perm_idx, axis=-2).reshape(B, 2, M, S)
  y_re = xp_re @ hr1 + xp_im @ hi1   (per block a of M)
  y_im = xp_re @ hi2 + xp_im @ hr2
  out  = stack([y_re, y_im], 1).reshape(B, 2, N, R)

Sharding: block dim M=1024 split across 8 cores (128 blocks each).
Permutation gather + all layout shuffles happen host-side in numpy.

Precision: weights are quantized to fp8 e3m4 (scaled x16 into the format's
normal range), x carries the inverse scale in fp16 (exact, power of two).
Measured end-to-end rel err 1.33e-2 against the fp32 reference (gate 2e-2).

Per-core device kernel, per block a:
  psum[16, 256]  = x_re[:, a].T @ [hr1[a] | hi2[a]]   (start)
                 + x_im[:, a].T @ [hi1[a] | hr2[a]]   (stop)
  -> cols 0:128 = y_re[a], cols 128:256 = y_im[a]; evicted to fp16 on
  alternating DVE/ACT.

Queue discipline: weight loads run alone on the Sync DGE queue (pure
stream, deep prefetch); y stores go on the Scalar DGE queue so a store
waiting on evictions can never block weight prefetch.
"""

import os
import numpy as np

B = 16
N = 4096
R = 32
M = 1024   # blocks
S = 128    # block size (contract dim)
NCORES = 8
MLOC = M // NCORES   # 128 blocks per core
# weight DMA group sizes in blocks (512 KiB per dma_start, 4 KiB
# per-partition segments — fat segments keep DMA packet efficiency high)
WGRPS = [8] * 16
# x chunk sizes in blocks (per re/im tensor); interleaved into the sync
# queue after the matching early w groups
XCHUNKS = [32, 32, 32, 32]
WSCALE = 16.0        # host multiplies W by this, divides x by it (exact)


def _prefix(sizes):
    out = [0]
    for n in sizes:
        out.append(out[-1] + n)
    return out


WG0 = _prefix(WGRPS)
XC0 = _prefix(XCHUNKS)
assert WG0[-1] == MLOC and XC0[-1] == MLOC
assert all(n % 2 == 0 for n in WGRPS)


def _xchunk_of(a):
    for c in range(len(XCHUNKS)):
        if a < XC0[c + 1]:
            return c
    raise AssertionError(a)


_NC_CACHE = {}


def _build_nc():
    import concourse.bacc as bacc
    import concourse.bass as bass
    import concourse.mybir as mybir
    from concourse import tile

    x_dt = mybir.dt.float16
    w_dt = mybir.dt.float8e3
    y_dt = mybir.dt.float16
    nc = bacc.Bacc(None, target_bir_lowering=False)

    xrp = nc.dram_tensor("xrp", [S, MLOC * B], x_dt, kind="ExternalInput")
    xip = nc.dram_tensor("xip", [S, MLOC * B], x_dt, kind="ExternalInput")
    # weights: per block 512 fp8 cols = [W1 | W2]
    # with W1 = [hr1 | hi2], W2 = [hi1 | hr2]  (x16 in e3m4)
    WC = 4 * S  # 512 cols per block
    w = nc.dram_tensor("w", [S, MLOC * WC], w_dt, kind="ExternalInput")
    y = nc.dram_tensor("y", [B, MLOC * 2 * S], y_dt, kind="ExternalOutput")

    with tile.TileContext(nc) as tc:
        with (
            tc.tile_pool(name="xp", bufs=1) as xpool,
            tc.tile_pool(name="wp", bufs=8) as wpool,
            tc.tile_pool(name="op", bufs=4) as opool,
            tc.tile_pool(name="ps", bufs=8, space=bass.MemorySpace.PSUM) as ps,
        ):
            # single load stream on the sync queue: w0, x0, w1, x1, ...,
            # then the remaining w groups. Stores live on the scalar queue
            # so they can never block weight prefetch.
            xr_t = [None] * len(XCHUNKS)
            xi_t = [None] * len(XCHUNKS)
            wts = []
            for g, nb in enumerate(WGRPS):
                wt = wpool.tile([S, nb * WC], w_dt)
                nc.sync.dma_start(wt[:], w[:, WG0[g] * WC:WG0[g + 1] * WC])
                wts.append(wt)
                if g < len(XCHUNKS):
                    cw = XCHUNKS[g] * B
                    xr_t[g] = xpool.tile([S, cw], x_dt, name=f"xr{g}")
                    xi_t[g] = xpool.tile([S, cw], x_dt, name=f"xi{g}")
                    nc.sync.dma_start(xr_t[g][:], xrp[:, XC0[g] * B:XC0[g + 1] * B])
                    nc.sync.dma_start(xi_t[g][:], xip[:, XC0[g] * B:XC0[g + 1] * B])
            npair = 0
            for g, nb in enumerate(WGRPS):
                wt = wts[g]
                ot = opool.tile([B, nb * 2 * S], y_dt)
                for i in range(0, nb, 2):
                    # two blocks share one full PSUM bank
                    pt = ps.tile([B, 4 * S], mybir.dt.float32)
                    for j in range(2):
                        a = WG0[g] + i + j
                        c0 = (i + j) * WC
                        w1 = wt[:, c0:c0 + 2 * S]
                        w2 = wt[:, c0 + 2 * S:c0 + 4 * S]
                        xc = _xchunk_of(a)
                        xo = a - XC0[xc]
                        xs = slice(xo * B, (xo + 1) * B)
                        psl = pt[:, j * 2 * S:(j + 1) * 2 * S]
                        nc.tensor.matmul(psl, xr_t[xc][:, xs], w1, start=True, stop=False)
                        nc.tensor.matmul(psl, xi_t[xc][:, xs], w2, start=False, stop=True)
                    osl = ot[:, i * 2 * S:(i + 2) * 2 * S]
                    if npair % 2 == 0:
                        nc.vector.tensor_copy(osl, pt[:])
                    else:
                        nc.scalar.copy(osl, pt[:])
                    npair += 1
                nc.scalar.dma_start(y[:, WG0[g] * 2 * S:WG0[g + 1] * 2 * S], ot[:])
    nc.compile()
    return nc


def kernel(x, hr1, hi1, hr2, hi2, perm_idx):
    import ml_dtypes
    from concourse.bass_utils import run_bass_kernel_spmd

    if "nc" not in _NC_CACHE:
        _NC_CACHE["nc"] = _build_nc()
    nc = _NC_CACHE["nc"]

    f8 = ml_dtypes.float8_e3m4
    x = np.asarray(x, dtype=np.float32)
    perm_idx = np.asarray(perm_idx)
    # host-side permutation gather + regroup into M blocks of size S
    xp = x[:, :, perm_idx, :].reshape(B, 2, M, S) * np.float32(1.0 / WSCALE)
    xp16 = xp.astype(np.float16)

    in_maps = []
    for c in range(NCORES):
        a0 = c * MLOC
        sl = slice(a0, a0 + MLOC)
        # [B, MLOC, S] -> [S(j), MLOC, B] -> [S, MLOC*B]
        xre = np.ascontiguousarray(
            np.transpose(xp16[:, 0, sl, :], (2, 1, 0))
        ).reshape(S, MLOC * B)
        xim = np.ascontiguousarray(
            np.transpose(xp16[:, 1, sl, :], (2, 1, 0))
        ).reshape(S, MLOC * B)
        # W1 = [hr1 | hi2], W2 = [hi1 | hr2]; per block [W1 | W2] x16 in e3m4
        wc = np.concatenate(
            [hr1[sl], hi2[sl], hi1[sl], hr2[sl]], axis=2
        ) * np.float32(WSCALE)
        wc = wc.astype(f8)  # [MLOC, S, 4S]
        wc = np.ascontiguousarray(np.transpose(wc, (1, 0, 2))).reshape(S, MLOC * 4 * S)
        in_maps.append({"xrp": xre, "xip": xim, "w": wc})

    trace = bool(os.environ.get("KERNEL_TRACE"))
    kwargs = {}
    if trace:
        kwargs["tmpdir"] = os.environ.get("KERNEL_TRACE_DIR") or None
    res = run_bass_kernel_spmd(nc, in_maps, core_ids=list(range(NCORES)), trace=trace, **kwargs)
    if trace and res.exec_time_ns is not None:
        print(f"HW exec time: {res.exec_time_ns} ns")
        _NC_CACHE["exec_time_ns"] = res.exec_time_ns
        _NC_CACHE["profile"] = res

    out = np.empty((B, 2, M, S), dtype=np.float32)
    for c in range(NCORES):
        a0 = c * MLOC
        yc = res.results[c]["y"].astype(np.float32).reshape(B, MLOC, 2, S)
        out[:, 0, a0:a0 + MLOC, :] = yc[:, :, 0, :]
        out[:, 1, a0:a0 + MLOC, :] = yc[:, :, 1, :]
    return out.reshape(B, 2, N, R)
